# revision 20
# baseline (speedup 1.0000x reference)
"""Trainium2 Bass kernel for causal multi-head attention (B=4, N=2048, DIM=1024, H=16, DH=64).

Sharding: 8 cores = (batch, head-group) pairs. Core c handles batch c//2 and
heads (c%2)*8 .. (c%2)*8+7.  Each core computes QKV projection for its 8 heads,
causal flash-attention, and a partial output projection (its heads' rows of
w_out).  The host sums the two partial outputs per batch and adds b_out.

Device-side layout choices (per core):
  - x is fed pre-transposed as xT [DIM, N] bf16 (host prep), so the QKV
    projection contraction (over DIM) sits on partitions with no on-device
    transpose.
  - Q^T, K^T computed as [head_dim, tok] (weights-stationary matmuls) so that
    scores can be computed directly as S^T = K^T.T @ Q^T with contraction dh=64.
  - S^T tiles are [128 k-tok, 512 q-tok]; softmax denominator comes free by
    augmenting V with a ones column: O^T_aug = [V | 1].T @ exp(S^T).
  - V computed as [tok, dh] (x-stationary matmuls), stored interleaved with the
    ones column: per k-tile [128, 8*65].
  - Causal masking: multiply exp(S^T) by precomputed 0/1 bf16 tiles on the
    diagonal blocks only (exp of a finite garbage score times 0 is exactly 0).
  - Key-padding mask folds into V_aug: V_aug row k scaled by mask[k] zeroes both
    numerator and denominator contributions of masked keys.
"""

import numpy as np
import ml_dtypes

B, N, DIM, H, DH = 4, 2048, 1024, 16, 64
HPC = 8            # heads per core
HD = HPC * DH      # 512 head dims per core
NCORES = 8
BF16 = ml_dtypes.bfloat16

TOK_TILE = 128     # k-token tile (partition dim of S^T)
QCHUNK = 512       # q-token chunk (free dim of S^T)
NKT = N // TOK_TILE       # 16 k tiles
NQC = N // QCHUNK         # 4 q chunks
NQT = N // 128            # 16 q tiles (out-projection)
DCH = DIM // 128          # 8 contraction chunks over DIM
VROW = HPC * (DH + 1)     # 520: V_aug row elems per k-tile

_CACHE = {}


def _build_program():
    from contextlib import ExitStack
    import concourse.bass as bass
    import concourse.tile as tile
    from concourse import bacc, mybir

    dt = mybir.dt
    f32 = dt.float32
    bf16 = dt.bfloat16
    Exp = mybir.ActivationFunctionType.Exp

    nc = bacc.Bacc("TRN2", target_bir_lowering=False, debug=False,
                   enable_asserts=False, num_devices=NCORES)

    xT = nc.dram_tensor("xT", [DIM, N], bf16, kind="ExternalInput").ap()
    wq = nc.dram_tensor("wq", [DIM, HD], bf16, kind="ExternalInput").ap()
    wk = nc.dram_tensor("wk", [DIM, HD], bf16, kind="ExternalInput").ap()
    wv = nc.dram_tensor("wv", [DIM, HD], bf16, kind="ExternalInput").ap()
    wo = nc.dram_tensor("wo", [HD, DIM], bf16, kind="ExternalInput").ap()
    kpm = nc.dram_tensor("kpm", [N, 1], f32, kind="ExternalInput").ap()
    cmask_d = nc.dram_tensor("cmask", [4 * 128, QCHUNK], bf16,
                             kind="ExternalInput").ap()
    out_d = nc.dram_tensor("out", [N, DIM], bf16, kind="ExternalOutput").ap()

    with tile.TileContext(nc) as tc, ExitStack() as ctx:
        const = ctx.enter_context(tc.tile_pool(name="const", bufs=1))
        p_sbp = ctx.enter_context(tc.tile_pool(name="p_sbp", bufs=6))
        miscp = ctx.enter_context(tc.tile_pool(name="miscp", bufs=3))
        outp = ctx.enter_context(tc.tile_pool(name="outp", bufs=3))
        mm_ps = ctx.enter_context(tc.tile_pool(name="mm_ps", bufs=2, space="PSUM"))
        s_ps = ctx.enter_context(tc.tile_pool(name="s_ps", bufs=2, space="PSUM"))
        o_ps = ctx.enter_context(tc.tile_pool(name="o_ps", bufs=2, space="PSUM"))

        # ---- persistent SBUF tensors (inputs merged into single tiles so
        # each loads with ONE strided DMA descriptor — the Sync engine
        # issues descriptors at only ~1.6/us, so descriptor count gates
        # the startup) ----
        XT = const.tile([128, DCH * N], bf16, name="XTsb")
        WQ = const.tile([128, DCH * HD], bf16, name="WQsb")
        WK = const.tile([128, DCH * HD], bf16, name="WKsb")
        WV = const.tile([128, DCH * HD], bf16, name="WVsb")
        WO = const.tile([128, 4 * DIM], bf16, name="WOsb")
        xT_sb = [XT[:, c * N:(c + 1) * N] for c in range(DCH)]
        wq_sb = [WQ[:, c * HD:(c + 1) * HD] for c in range(DCH)]
        wk_sb = [WK[:, c * HD:(c + 1) * HD] for c in range(DCH)]
        wv_sb = [WV[:, c * HD:(c + 1) * HD] for c in range(DCH)]
        wo_sb = [WO[:, c * DIM:(c + 1) * DIM] for c in range(4)]
        # Q^T / K^T packed: chunk c holds heads 2c (parts 0-63) and 2c+1 (64-127)
        QT = [const.tile([128, N], bf16, name=f"QTsb{c}") for c in range(4)]
        KT = [const.tile([128, N], bf16, name=f"KTsb{c}") for c in range(4)]
        # V_aug: per k-tile block of 8*(64+1) cols
        V = const.tile([128, NKT * VROW], bf16, name="Vsb")
        # O^T packed like QT/KT
        OT = [const.tile([128, N], bf16, name=f"OTsb{c}") for c in range(4)]
        cmask = const.tile([128, 4 * QCHUNK], bf16, name="cmasksb")
        # key-padding mask: col t = mask[t*128 + p] (one tiny DMA, loaded
        # first so V-proj evacuations never wait behind the big weight loads)
        kpm_sb = const.tile([128, NKT], f32, name="kpmsb")

        sync = nc.sync
        sync.dma_start(
            kpm_sb.rearrange("p (t one) -> p t one", one=1),
            kpm.rearrange("(t p) one -> p t one", p=128),
        )

        # ---- load inputs: wv + xT first 256 cols first so the first v_proj
        # accumulation groups start early; each pass is ONE descriptor ----
        xT_src = xT.rearrange("(c p) n -> p c n", p=128)
        XT3 = XT.rearrange("p (c n) -> p c n", n=N)
        wv_src = wv.rearrange("(c p) h -> p c h", p=128)
        WV3 = WV.rearrange("p (c h) -> p c h", h=HD)
        # finer granularity on the critical first pieces so the first v_proj
        # accumulation group streams in as chunks land (it contracts chunks
        # in order, so chunk 0-1 arriving first lets the group start)
        sync.dma_start(WV3[:, 0:2], wv_src[:, 0:2])
        sync.dma_start(XT3[:, 0:2, 0:256], xT_src[:, 0:2, 0:256])
        sync.dma_start(WV3[:, 2:8], wv_src[:, 2:8])
        sync.dma_start(XT3[:, 2:8, 0:256], xT_src[:, 2:8, 0:256])
        sync.dma_start(XT3[:, :, 256:N // 2], xT_src[:, :, 256:N // 2])
        sync.dma_start(XT3[:, :, N // 2:N], xT_src[:, :, N // 2:N])
        sync.dma_start(WQ.rearrange("p (c h) -> p c h", h=HD),
                       wq.rearrange("(c p) h -> p c h", p=128))
        sync.dma_start(WK.rearrange("p (c h) -> p c h", h=HD),
                       wk.rearrange("(c p) h -> p c h", p=128))
        sync.dma_start(WO.rearrange("p (c d) -> p c d", d=DIM),
                       wo.rearrange("(c p) d -> p c d", p=128))
        # cmask DRAM row r*128+k, col q  ->  SBUF part k, col r*512+q
        sync.dma_start(
            cmask.rearrange("p (r q) -> p r q", r=4),
            cmask_d.rearrange("(r p) q -> p r q", p=128),
        )

        # ---- V projection: V[tok, dh] via x-stationary matmuls ----
        def v_proj(half):
            for kt in range(half * (NKT // 2), (half + 1) * (NKT // 2)):
                kpm_t = kpm_sb[:, kt:kt + 1]
                ps = mm_ps.tile([128, 512], f32, tag="mm", name="ps")
                for c in range(DCH):
                    nc.tensor.matmul(
                        ps[:], xT_sb[c][:, kt * 128:(kt + 1) * 128],
                        wv_sb[c][:],
                        start=(c == 0), stop=(c == DCH - 1))
                vblk = V[:, kt * VROW:(kt + 1) * VROW].rearrange(
                    "p (h c) -> p h c", c=DH + 1)
                # data cols, scaled by key-padding mask
                nc.vector.tensor_scalar_mul(
                    vblk[:, :, 0:DH],
                    ps.rearrange("p (h c) -> p h c", c=DH),
                    kpm_t[:, 0:1])
                # ones column = mask value (free-dim stride-0 broadcast read)
                nc.vector.tensor_copy(vblk[:, :, DH:DH + 1].squeeze(),
                                      kpm_t[:, 0:1].broadcast_to([128, HPC]))

        def qk_proj(c):
            for tcx in range(NQC):
                tsl = slice(tcx * QCHUNK, (tcx + 1) * QCHUNK)
                psq = mm_ps.tile([128, 512], f32, tag="mm", name="psq")
                for d in range(DCH):
                    nc.tensor.matmul(
                        psq[:], wq_sb[d][:, c * 128:(c + 1) * 128],
                        xT_sb[d][:, tsl],
                        start=(d == 0), stop=(d == DCH - 1))
                nc.vector.tensor_copy(QT[c][:, tsl], psq[:])
                psk = mm_ps.tile([128, 512], f32, tag="mm", name="psk")
                for d in range(DCH):
                    nc.tensor.matmul(
                        psk[:], wk_sb[d][:, c * 128:(c + 1) * 128],
                        xT_sb[d][:, tsl],
                        start=(d == 0), stop=(d == DCH - 1))
                nc.vector.tensor_copy(KT[c][:, tsl], psk[:])

        def attend(h, qc):
            c = h // 2
            po = (h % 2) * 64          # partition offset within chunk
            qt_h = QT[c][po:po + 64, :]
            kt_h = KT[c][po:po + 64, :]
            qsl = slice(qc * QCHUNK, (qc + 1) * QCHUNK)
            pso = o_ps.tile([DH + 1, 512], f32, tag="o", name="pso")
            nkt = 4 * qc + 4
            for kp in range(nkt // 2):
                ps2 = s_ps.tile([128, 1024], f32, tag="s", name="ps2")
                r = 2 * kp - 4 * qc
                for j in (0, 1):
                    kt = 2 * kp + j
                    # diagonal k-tile: q < (r+j)*128 fully masked -> narrow
                    off = max(0, (kt - 4 * qc) * 128)
                    nc.tensor.matmul(
                        ps2[:, j * 512 + off:(j + 1) * 512],
                        kt_h[:, kt * 128:(kt + 1) * 128],
                        qt_h[:, qc * QCHUNK + off:(qc + 1) * QCHUNK],
                        start=True, stop=True)
                p2 = p_sbp.tile([128, 1024], bf16, tag="p", name="p2")
                if r >= 0:
                    # per-half exp + causal mask over only the written cols
                    for j in (0, 1):
                        off = (r + j) * 128
                        sl = slice(j * 512 + off, (j + 1) * 512)
                        nc.scalar.activation(p2[:, sl], ps2[:, sl], Exp)
                        nc.vector.tensor_mul(
                            p2[:, sl], p2[:, sl],
                            cmask[:, (r + j) * QCHUNK + off:
                                  (r + j + 1) * QCHUNK])
                else:
                    nc.scalar.activation(p2[:], ps2[:], Exp)
                for j in (0, 1):
                    kt = 2 * kp + j
                    off = max(0, (kt - 4 * qc) * 128)
                    nc.tensor.matmul(
                        pso[:, off:512],
                        V[:, kt * VROW + h * (DH + 1):
                           kt * VROW + (h + 1) * (DH + 1)],
                        p2[:, j * 512 + off:(j + 1) * 512],
                        start=(kt == 0), stop=(kt == nkt - 1),
                        skip_group_check=True)
            # normalize: O^T[0:64] * (1 / rowsum row 64)
            # (stage rowsum into SBUF: custom-DVE recip needs SBUF in)
            rsum = miscp.tile([1, 512], f32, tag="rsum", name="rsum")
            nc.vector.tensor_copy(rsum[:], pso[DH:DH + 1, :])
            recip = miscp.tile([1, 512], f32, tag="recip", name="recip")
            nc.vector.reciprocal_approx_fast(recip[:], rsum[:])
            bcast = miscp.tile([64, 512], f32, tag="bcast", name="bcast")
            nc.gpsimd.partition_broadcast(bcast[:], recip[:])
            if po == 0:
                nc.vector.tensor_mul(OT[c][0:64, qsl],
                                     pso[0:DH, :], bcast[:])
            else:
                otmp = miscp.tile([64, 512], bf16, tag="otmp", bufs=3,
                                  name="otmp")
                nc.vector.tensor_mul(otmp[:], pso[0:DH, :], bcast[:])
                # partition shift 0->64 needs a DMA, engines can't shift
                sync.dma_start(OT[c][64:128, qsl], otmp[:])

        def out_proj(qt, tail=False):
            y_sb = outp.tile([128, DIM], bf16, tag="y", name="y_sb")
            for oc in range(2):
                psy = mm_ps.tile([128, 512], f32, tag="mm", name="psy")
                for cc in range(4):
                    nc.tensor.matmul(
                        psy[:], OT[cc][:, qt * 128:(qt + 1) * 128],
                        wo_sb[cc][:, oc * 512:(oc + 1) * 512],
                        start=(cc == 0), stop=(cc == 3))
                # in the tail the Scalar engine is idle (no exps left), so
                # evacuate there and keep DVE free for the normalize chain;
                # per-oc DMA starts the last transfers earlier
                if tail:
                    nc.scalar.copy(y_sb[:, oc * 512:(oc + 1) * 512], psy[:])
                else:
                    nc.vector.tensor_copy(y_sb[:, oc * 512:(oc + 1) * 512],
                                          psy[:])
                # in the tail, alternate the writes between the Sync and
                # Scalar DGE rings so the final drain uses two queues
                dge = nc.scalar if (tail and oc == 1) else sync
                dge.dma_start(
                    out_d[qt * 128:(qt + 1) * 128, oc * 512:(oc + 1) * 512],
                    y_sb[:, oc * 512:(oc + 1) * 512])

        # ---- proj chunk c then its two heads (overlaps ACT exp with PE
        # proj of later chunks); last pair goes qc-major (descending, so
        # the final chain is the short qc=0) with inline Y ----
        v_proj(0)
        v_proj(1)
        qk_proj(0)
        for h in (0, 1):
            for qc in range(NQC):
                attend(h, qc)
        for c in (1, 2):
            qk_proj(c)
            for h in (2 * c, 2 * c + 1):
                for qc in range(NQC):
                    attend(h, qc)
        # last pair: qc descending, with each out_proj group delayed one qc
        # step so its OT dependency (normalize chain) is long since resolved
        # when the PE reaches it — no head-of-line blocking in the strict
        # in-order PE queue
        qk_proj(3)
        for qc in (3, 2, 1, 0):
            attend(7, qc)
            attend(6, qc)
            if qc < 3:
                for qt in range(4 * (qc + 1), 4 * (qc + 1) + 4):
                    out_proj(qt, tail=(qc == 0))
        for qt in range(0, 4):
            out_proj(qt, tail=True)

    nc.compile()
    return nc


def _get_program():
    if "nc" not in _CACHE:
        _CACHE["nc"] = _build_program()
    return _CACHE["nc"]


def _prep_inputs(x, mask, w_qkv, w_out):
    """Build the 8 per-core input maps (host-side sharding)."""
    scale = DH ** -0.5
    # causal keep-mask patterns for the 4 diagonal k-tiles of a 512 q-chunk
    k_idx = np.arange(128)[:, None]
    q_idx = np.arange(QCHUNK)[None, :]
    cm = np.concatenate(
        [(q_idx >= r * 128 + k_idx) for r in range(4)], axis=0
    ).astype(BF16)  # [512, 512]

    xT = [np.ascontiguousarray(x[b].T).astype(BF16) for b in range(B)]
    in_maps = []
    for core in range(NCORES):
        b, hg = core // 2, core % 2
        cs = slice(hg * HD, (hg + 1) * HD)
        wq_s = (w_qkv[:, 0 * DIM:1 * DIM][:, cs] * scale).astype(BF16)
        wk_s = w_qkv[:, 1 * DIM:2 * DIM][:, cs].astype(BF16)
        wv_s = w_qkv[:, 2 * DIM:3 * DIM][:, cs].astype(BF16)
        wo_s = np.ascontiguousarray(w_out[cs, :]).astype(BF16)
        kpm = mask[b].astype(np.float32).reshape(N, 1)
        in_maps.append({
            "xT": xT[b], "wq": wq_s, "wk": wk_s, "wv": wv_s, "wo": wo_s,
            "kpm": np.ascontiguousarray(kpm), "cmask": cm,
        })
    return in_maps


def kernel(x, mask, w_qkv, w_out, b_out, _trace=False):
    from concourse import bass_utils

    x = np.asarray(x, dtype=np.float32)
    mask = np.asarray(mask)
    w_qkv = np.asarray(w_qkv, dtype=np.float32)
    w_out = np.asarray(w_out, dtype=np.float32)
    b_out = np.asarray(b_out, dtype=np.float32)

    nc = _get_program()
    in_maps = _prep_inputs(x, mask, w_qkv, w_out)
    res = bass_utils.run_bass_kernel_spmd(
        nc, in_maps, core_ids=list(range(NCORES)), trace=_trace)

    out = np.empty((B, N, DIM), dtype=np.float32)
    for b in range(B):
        out[b] = (res.results[2 * b]["out"].astype(np.float32)
                  + res.results[2 * b + 1]["out"].astype(np.float32) + b_out)
    if _trace:
        return out, res
    return out



# revision 22
# speedup vs baseline: 1.0044x; 1.0044x over previous
"""Trainium2 Bass kernel for causal multi-head attention (B=4, N=2048, DIM=1024, H=16, DH=64).

Sharding: 8 cores = (batch, head-group) pairs. Core c handles batch c//2 and
heads (c%2)*8 .. (c%2)*8+7.  Each core computes QKV projection for its 8 heads,
causal flash-attention, and a partial output projection (its heads' rows of
w_out).  The host sums the two partial outputs per batch and adds b_out.

Device-side layout choices (per core):
  - x is fed pre-transposed as xT [DIM, N] bf16 (host prep), so the QKV
    projection contraction (over DIM) sits on partitions with no on-device
    transpose.
  - Q^T, K^T computed as [head_dim, tok] (weights-stationary matmuls) so that
    scores can be computed directly as S^T = K^T.T @ Q^T with contraction dh=64.
  - S^T tiles are [128 k-tok, 512 q-tok]; softmax denominator comes free by
    augmenting V with a ones column: O^T_aug = [V | 1].T @ exp(S^T).
  - V computed as [tok, dh] (x-stationary matmuls), stored interleaved with the
    ones column: per k-tile [128, 8*65].
  - Causal masking: multiply exp(S^T) by precomputed 0/1 bf16 tiles on the
    diagonal blocks only (exp of a finite garbage score times 0 is exactly 0).
  - Key-padding mask folds into V_aug: V_aug row k scaled by mask[k] zeroes both
    numerator and denominator contributions of masked keys.
"""

import numpy as np
import ml_dtypes

B, N, DIM, H, DH = 4, 2048, 1024, 16, 64
HPC = 8            # heads per core
HD = HPC * DH      # 512 head dims per core
NCORES = 8
BF16 = ml_dtypes.bfloat16

TOK_TILE = 128     # k-token tile (partition dim of S^T)
QCHUNK = 512       # q-token chunk (free dim of S^T)
NKT = N // TOK_TILE       # 16 k tiles
NQC = N // QCHUNK         # 4 q chunks
NQT = N // 128            # 16 q tiles (out-projection)
DCH = DIM // 128          # 8 contraction chunks over DIM
VROW = HPC * (DH + 1)     # 520: V_aug row elems per k-tile

_CACHE = {}


def _build_program():
    from contextlib import ExitStack
    import concourse.bass as bass
    import concourse.tile as tile
    from concourse import bacc, mybir

    dt = mybir.dt
    f32 = dt.float32
    bf16 = dt.bfloat16
    Exp = mybir.ActivationFunctionType.Exp

    nc = bacc.Bacc("TRN2", target_bir_lowering=False, debug=False,
                   enable_asserts=False, num_devices=NCORES)

    xT = nc.dram_tensor("xT", [DIM, N], bf16, kind="ExternalInput").ap()
    wq = nc.dram_tensor("wq", [DIM, HD], bf16, kind="ExternalInput").ap()
    wk = nc.dram_tensor("wk", [DIM, HD], bf16, kind="ExternalInput").ap()
    wv = nc.dram_tensor("wv", [DIM, HD], bf16, kind="ExternalInput").ap()
    wo = nc.dram_tensor("wo", [HD, DIM], bf16, kind="ExternalInput").ap()
    kpm = nc.dram_tensor("kpm", [N, 1], f32, kind="ExternalInput").ap()
    cmask_d = nc.dram_tensor("cmask", [4 * 128, QCHUNK], bf16,
                             kind="ExternalInput").ap()
    out_d = nc.dram_tensor("out", [N, DIM], bf16, kind="ExternalOutput").ap()

    with tile.TileContext(nc) as tc, ExitStack() as ctx:
        const = ctx.enter_context(tc.tile_pool(name="const", bufs=1))
        p_sbp = ctx.enter_context(tc.tile_pool(name="p_sbp", bufs=6))
        miscp = ctx.enter_context(tc.tile_pool(name="miscp", bufs=3))
        outp = ctx.enter_context(tc.tile_pool(name="outp", bufs=3))
        mm_ps = ctx.enter_context(tc.tile_pool(name="mm_ps", bufs=2, space="PSUM"))
        s_ps = ctx.enter_context(tc.tile_pool(name="s_ps", bufs=2, space="PSUM"))
        o_ps = ctx.enter_context(tc.tile_pool(name="o_ps", bufs=2, space="PSUM"))

        # ---- persistent SBUF tensors (inputs merged into single tiles so
        # each loads with ONE strided DMA descriptor — the Sync engine
        # issues descriptors at only ~1.6/us, so descriptor count gates
        # the startup) ----
        XT = const.tile([128, DCH * N], bf16, name="XTsb")
        WQ = const.tile([128, DCH * HD], bf16, name="WQsb")
        WK = const.tile([128, DCH * HD], bf16, name="WKsb")
        WV = const.tile([128, DCH * HD], bf16, name="WVsb")
        WO = const.tile([128, 4 * DIM], bf16, name="WOsb")
        xT_sb = [XT[:, c * N:(c + 1) * N] for c in range(DCH)]
        wq_sb = [WQ[:, c * HD:(c + 1) * HD] for c in range(DCH)]
        wk_sb = [WK[:, c * HD:(c + 1) * HD] for c in range(DCH)]
        wv_sb = [WV[:, c * HD:(c + 1) * HD] for c in range(DCH)]
        wo_sb = [WO[:, c * DIM:(c + 1) * DIM] for c in range(4)]
        # Q^T / K^T packed: chunk c holds heads 2c (parts 0-63) and 2c+1 (64-127)
        QT = [const.tile([128, N], bf16, name=f"QTsb{c}") for c in range(4)]
        KT = [const.tile([128, N], bf16, name=f"KTsb{c}") for c in range(4)]
        # V_aug: per k-tile block of 8*(64+1) cols
        V = const.tile([128, NKT * VROW], bf16, name="Vsb")
        # O^T packed like QT/KT
        OT = [const.tile([128, N], bf16, name=f"OTsb{c}") for c in range(4)]
        cmask = const.tile([128, 4 * QCHUNK], bf16, name="cmasksb")
        # key-padding mask: col t = mask[t*128 + p] (one tiny DMA, loaded
        # first so V-proj evacuations never wait behind the big weight loads)
        kpm_sb = const.tile([128, NKT], f32, name="kpmsb")

        sync = nc.sync
        sync.dma_start(
            kpm_sb.rearrange("p (t one) -> p t one", one=1),
            kpm.rearrange("(t p) one -> p t one", p=128),
        )

        # ---- load inputs: wv + xT first 256 cols first so the first v_proj
        # accumulation groups start early; each pass is ONE descriptor ----
        xT_src = xT.rearrange("(c p) n -> p c n", p=128)
        XT3 = XT.rearrange("p (c n) -> p c n", n=N)
        wv_src = wv.rearrange("(c p) h -> p c h", p=128)
        WV3 = WV.rearrange("p (c h) -> p c h", h=HD)
        # finer granularity on the critical first pieces so the first v_proj
        # accumulation group streams in as chunks land (it contracts chunks
        # in order, so chunk 0-1 arriving first lets the group start)
        sync.dma_start(WV3[:, 0:4], wv_src[:, 0:4])
        sync.dma_start(XT3[:, 0:4, 0:256], xT_src[:, 0:4, 0:256])
        sync.dma_start(WV3[:, 4:8], wv_src[:, 4:8])
        sync.dma_start(XT3[:, 4:8, 0:256], xT_src[:, 4:8, 0:256])
        sync.dma_start(XT3[:, :, 256:N // 2], xT_src[:, :, 256:N // 2])
        sync.dma_start(XT3[:, :, N // 2:N], xT_src[:, :, N // 2:N])
        sync.dma_start(WQ.rearrange("p (c h) -> p c h", h=HD),
                       wq.rearrange("(c p) h -> p c h", p=128))
        sync.dma_start(WK.rearrange("p (c h) -> p c h", h=HD),
                       wk.rearrange("(c p) h -> p c h", p=128))
        sync.dma_start(WO.rearrange("p (c d) -> p c d", d=DIM),
                       wo.rearrange("(c p) d -> p c d", p=128))
        # cmask DRAM row r*128+k, col q  ->  SBUF part k, col r*512+q
        sync.dma_start(
            cmask.rearrange("p (r q) -> p r q", r=4),
            cmask_d.rearrange("(r p) q -> p r q", p=128),
        )

        # ---- V projection: V[tok, dh] via x-stationary matmuls ----
        def v_proj(half):
            for kt in range(half * (NKT // 2), (half + 1) * (NKT // 2)):
                kpm_t = kpm_sb[:, kt:kt + 1]
                ps = mm_ps.tile([128, 512], f32, tag="mm", name="ps")
                for c in range(DCH):
                    nc.tensor.matmul(
                        ps[:], xT_sb[c][:, kt * 128:(kt + 1) * 128],
                        wv_sb[c][:],
                        start=(c == 0), stop=(c == DCH - 1))
                vblk = V[:, kt * VROW:(kt + 1) * VROW].rearrange(
                    "p (h c) -> p h c", c=DH + 1)
                # data cols, scaled by key-padding mask
                nc.vector.tensor_scalar_mul(
                    vblk[:, :, 0:DH],
                    ps.rearrange("p (h c) -> p h c", c=DH),
                    kpm_t[:, 0:1])
                # ones column = mask value (free-dim stride-0 broadcast read)
                nc.vector.tensor_copy(vblk[:, :, DH:DH + 1].squeeze(),
                                      kpm_t[:, 0:1].broadcast_to([128, HPC]))

        def qk_proj(c):
            for tcx in range(NQC):
                tsl = slice(tcx * QCHUNK, (tcx + 1) * QCHUNK)
                psq = mm_ps.tile([128, 512], f32, tag="mm", name="psq")
                for d in range(DCH):
                    nc.tensor.matmul(
                        psq[:], wq_sb[d][:, c * 128:(c + 1) * 128],
                        xT_sb[d][:, tsl],
                        start=(d == 0), stop=(d == DCH - 1))
                nc.vector.tensor_copy(QT[c][:, tsl], psq[:])
                psk = mm_ps.tile([128, 512], f32, tag="mm", name="psk")
                for d in range(DCH):
                    nc.tensor.matmul(
                        psk[:], wk_sb[d][:, c * 128:(c + 1) * 128],
                        xT_sb[d][:, tsl],
                        start=(d == 0), stop=(d == DCH - 1))
                nc.vector.tensor_copy(KT[c][:, tsl], psk[:])

        def attend(h, qc):
            c = h // 2
            po = (h % 2) * 64          # partition offset within chunk
            qt_h = QT[c][po:po + 64, :]
            kt_h = KT[c][po:po + 64, :]
            qsl = slice(qc * QCHUNK, (qc + 1) * QCHUNK)
            pso = o_ps.tile([DH + 1, 512], f32, tag="o", name="pso")
            nkt = 4 * qc + 4
            for kp in range(nkt // 2):
                ps2 = s_ps.tile([128, 1024], f32, tag="s", name="ps2")
                r = 2 * kp - 4 * qc
                for j in (0, 1):
                    kt = 2 * kp + j
                    # diagonal k-tile: q < (r+j)*128 fully masked -> narrow
                    off = max(0, (kt - 4 * qc) * 128)
                    nc.tensor.matmul(
                        ps2[:, j * 512 + off:(j + 1) * 512],
                        kt_h[:, kt * 128:(kt + 1) * 128],
                        qt_h[:, qc * QCHUNK + off:(qc + 1) * QCHUNK],
                        start=True, stop=True)
                p2 = p_sbp.tile([128, 1024], bf16, tag="p", name="p2")
                if r >= 0:
                    # per-half exp + causal mask over only the written cols
                    for j in (0, 1):
                        off = (r + j) * 128
                        sl = slice(j * 512 + off, (j + 1) * 512)
                        nc.scalar.activation(p2[:, sl], ps2[:, sl], Exp)
                        nc.vector.tensor_mul(
                            p2[:, sl], p2[:, sl],
                            cmask[:, (r + j) * QCHUNK + off:
                                  (r + j + 1) * QCHUNK])
                else:
                    nc.scalar.activation(p2[:], ps2[:], Exp)
                for j in (0, 1):
                    kt = 2 * kp + j
                    off = max(0, (kt - 4 * qc) * 128)
                    nc.tensor.matmul(
                        pso[:, off:512],
                        V[:, kt * VROW + h * (DH + 1):
                           kt * VROW + (h + 1) * (DH + 1)],
                        p2[:, j * 512 + off:(j + 1) * 512],
                        start=(kt == 0), stop=(kt == nkt - 1),
                        skip_group_check=True)
            # normalize: O^T[0:64] * (1 / rowsum row 64)
            # (stage rowsum into SBUF: custom-DVE recip needs SBUF in)
            rsum = miscp.tile([1, 512], f32, tag="rsum", name="rsum")
            nc.vector.tensor_copy(rsum[:], pso[DH:DH + 1, :])
            recip = miscp.tile([1, 512], f32, tag="recip", name="recip")
            nc.vector.reciprocal_approx_fast(recip[:], rsum[:])
            bcast = miscp.tile([64, 512], f32, tag="bcast", name="bcast")
            nc.gpsimd.partition_broadcast(bcast[:], recip[:])
            if po == 0:
                nc.vector.tensor_mul(OT[c][0:64, qsl],
                                     pso[0:DH, :], bcast[:])
            else:
                otmp = miscp.tile([64, 512], bf16, tag="otmp", bufs=3,
                                  name="otmp")
                nc.vector.tensor_mul(otmp[:], pso[0:DH, :], bcast[:])
                # partition shift 0->64 needs a DMA, engines can't shift
                sync.dma_start(OT[c][64:128, qsl], otmp[:])

        def out_proj(qt, tail=False):
            y_sb = outp.tile([128, DIM], bf16, tag="y", name="y_sb")
            for oc in range(2):
                psy = mm_ps.tile([128, 512], f32, tag="mm", name="psy")
                for cc in range(4):
                    nc.tensor.matmul(
                        psy[:], OT[cc][:, qt * 128:(qt + 1) * 128],
                        wo_sb[cc][:, oc * 512:(oc + 1) * 512],
                        start=(cc == 0), stop=(cc == 3))
                # in the tail the Scalar engine is idle (no exps left), so
                # evacuate there and keep DVE free for the normalize chain;
                # per-oc DMA starts the last transfers earlier
                if tail:
                    nc.scalar.copy(y_sb[:, oc * 512:(oc + 1) * 512], psy[:])
                else:
                    nc.vector.tensor_copy(y_sb[:, oc * 512:(oc + 1) * 512],
                                          psy[:])
                sync.dma_start(
                    out_d[qt * 128:(qt + 1) * 128, oc * 512:(oc + 1) * 512],
                    y_sb[:, oc * 512:(oc + 1) * 512])

        # ---- proj chunk c then its two heads (overlaps ACT exp with PE
        # proj of later chunks); last pair goes qc-major (descending, so
        # the final chain is the short qc=0) with inline Y ----
        v_proj(0)
        v_proj(1)
        qk_proj(0)
        for h in (0, 1):
            for qc in range(NQC):
                attend(h, qc)
        for c in (1, 2):
            qk_proj(c)
            for h in (2 * c, 2 * c + 1):
                for qc in range(NQC):
                    attend(h, qc)
        # last pair: qc descending, with each out_proj group delayed one qc
        # step so its OT dependency (normalize chain) is long since resolved
        # when the PE reaches it — no head-of-line blocking in the strict
        # in-order PE queue
        qk_proj(3)
        for qc in (3, 2, 1, 0):
            attend(7, qc)
            attend(6, qc)
            if qc < 3:
                for qt in range(4 * (qc + 1), 4 * (qc + 1) + 4):
                    out_proj(qt, tail=(qc == 0))
        for qt in range(0, 4):
            out_proj(qt, tail=True)

    nc.compile()
    return nc


def _get_program():
    if "nc" not in _CACHE:
        _CACHE["nc"] = _build_program()
    return _CACHE["nc"]


def _prep_inputs(x, mask, w_qkv, w_out):
    """Build the 8 per-core input maps (host-side sharding)."""
    scale = DH ** -0.5
    # causal keep-mask patterns for the 4 diagonal k-tiles of a 512 q-chunk
    k_idx = np.arange(128)[:, None]
    q_idx = np.arange(QCHUNK)[None, :]
    cm = np.concatenate(
        [(q_idx >= r * 128 + k_idx) for r in range(4)], axis=0
    ).astype(BF16)  # [512, 512]

    xT = [np.ascontiguousarray(x[b].T).astype(BF16) for b in range(B)]
    in_maps = []
    for core in range(NCORES):
        b, hg = core // 2, core % 2
        cs = slice(hg * HD, (hg + 1) * HD)
        wq_s = (w_qkv[:, 0 * DIM:1 * DIM][:, cs] * scale).astype(BF16)
        wk_s = w_qkv[:, 1 * DIM:2 * DIM][:, cs].astype(BF16)
        wv_s = w_qkv[:, 2 * DIM:3 * DIM][:, cs].astype(BF16)
        wo_s = np.ascontiguousarray(w_out[cs, :]).astype(BF16)
        kpm = mask[b].astype(np.float32).reshape(N, 1)
        in_maps.append({
            "xT": xT[b], "wq": wq_s, "wk": wk_s, "wv": wv_s, "wo": wo_s,
            "kpm": np.ascontiguousarray(kpm), "cmask": cm,
        })
    return in_maps


def kernel(x, mask, w_qkv, w_out, b_out, _trace=False):
    from concourse import bass_utils

    x = np.asarray(x, dtype=np.float32)
    mask = np.asarray(mask)
    w_qkv = np.asarray(w_qkv, dtype=np.float32)
    w_out = np.asarray(w_out, dtype=np.float32)
    b_out = np.asarray(b_out, dtype=np.float32)

    nc = _get_program()
    in_maps = _prep_inputs(x, mask, w_qkv, w_out)
    res = bass_utils.run_bass_kernel_spmd(
        nc, in_maps, core_ids=list(range(NCORES)), trace=_trace)

    out = np.empty((B, N, DIM), dtype=np.float32)
    for b in range(B):
        out[b] = (res.results[2 * b]["out"].astype(np.float32)
                  + res.results[2 * b + 1]["out"].astype(np.float32) + b_out)
    if _trace:
        return out, res
    return out

